# revision 12
# baseline (speedup 1.0000x reference)
"""Trainium2 Bass kernel for a single-token transformer decode block.

Model: B=2, S=8192, H=32, HD=128, D=4096, FF=11008 (gated SiLU MLP), RoPE,
RMSNorm, KV-cache decode at runtime position `pos`.

Sharding (tensor-parallel over heads, 8 cores):
  - core c owns heads 4c..4c+3: Wqkv rows, out_proj columns, k/v cache head
    slices, and an FF shard (1376 padded to 1408) of l1/l2/l3.
  - AllReduce after out_proj and after l3.
  - norms are folded into the weight matrices on the host; RoPE rows at `pos`
    are computed on the host (position is known when kernel() is called, so
    the device program is specialized on it: only ceil(pos/128) KV chunks are
    ever read or attended).

Layout strategy on device (everything fp32):
  - activations live as [B=2, features] on partitions 0..1 ("b-part") except
    where a matmul needs features-on-partitions; those are produced either by
    host-side transposes (weights, xT) or cheap PE transposes (h, m, q/k/v).
  - scores for one (b,h) live as [128 (s within chunk), n_chunks] in PSUM,
    computed by per-chunk matmuls lhsT=K^T[128hd,128s], rhs=q^T[128,1].
  - softmax skips the max-subtraction (scores are O(5) here; exp is safe in
    fp32 and matches the reference's -10000 masking, which underflows to 0).
  - the s==pos entry (new k/v) is handled as a separate rank-1 term.
"""

import math
import os
import sys
import types

import numpy as np

B = 2
S = 8192
H = 32
HD = 128
D = 4096
FF = 11008
P = 128
N_CORES = 8
NH = H // N_CORES            # 4 heads per core
DLOC = NH * HD               # 512
FFL = FF // N_CORES          # 1376
FFP = 1408                   # FFL padded to a multiple of 128
KC = D // P                  # 32 k-chunks of the model dim
FFC = FFP // P               # 11
EPS = 1e-6
SBLK = 16                    # s-chunks (of 128) per KV DMA block = 2048 positions

LAST_RESULT = None           # BassKernelResults of the most recent run
_NC_CACHE = {}


def _install_ntff_shim():
    """Make trace=True work under axon when antenv.axon_hooks is missing."""
    try:
        import antenv.axon_hooks  # noqa: F401
        return
    except ImportError:
        pass
    try:
        from trn_agent_boot.trn_boot import _ntff_profile_via_ctypes
        mod = types.ModuleType("antenv.axon_hooks")
        _hook = [None]
        mod.set_axon_ntff_profile_hook = lambda h: _hook.__setitem__(0, h)
        mod.get_axon_ntff_profile_hook = lambda: _hook[0]
        sys.modules["antenv.axon_hooks"] = mod
        mod.set_axon_ntff_profile_hook(
            _ntff_profile_via_ctypes("/opt/axon/libaxon_pjrt.so"))
    except Exception:
        pass


def _build(pos: int, dbg: bool = False):
    import concourse.mybir as mybir
    import concourse.tile as tile
    from concourse import bacc
    from concourse.masks import make_identity

    f32 = mybir.dt.float32
    Alu = mybir.AluOpType
    Act = mybir.ActivationFunctionType
    AX = mybir.AxisListType

    SC = (pos + P - 1) // P          # 128-chunks covering cache positions [0, pos)
    SCP = SC * P

    nc = bacc.Bacc("TRN2", target_bir_lowering=False, debug=False,
                   enable_asserts=False, num_devices=N_CORES)

    # ---- I/O -------------------------------------------------------------
    x_in = nc.dram_tensor("x", [B, D], f32, kind="ExternalInput")
    xT_in = nc.dram_tensor("xT", [P, KC, B], f32, kind="ExternalInput")
    wqkvT = nc.dram_tensor("wqkvT", [D, 3 * DLOC], f32, kind="ExternalInput")
    opT = nc.dram_tensor("opT", [DLOC, D], f32, kind="ExternalInput")
    l1T = nc.dram_tensor("l1T", [D, FFP], f32, kind="ExternalInput")
    l2T = nc.dram_tensor("l2T", [D, FFP], f32, kind="ExternalInput")
    l3T = nc.dram_tensor("l3T", [FFP, D], f32, kind="ExternalInput")
    ropeq = nc.dram_tensor("ropeq", [B, HD], f32, kind="ExternalInput")
    ropek = nc.dram_tensor("ropek", [B, HD], f32, kind="ExternalInput")
    if SC > 0:
        kT_in = nc.dram_tensor("kT", [B, NH, HD, SCP], f32, kind="ExternalInput")
        v_in = nc.dram_tensor("v", [B, SCP, NH, HD], f32, kind="ExternalInput")
        mask_in = nc.dram_tensor("mask", [P, SC], f32, kind="ExternalInput")

    x_out = nc.dram_tensor("x_out", [B, D], f32, kind="ExternalOutput")
    k_row = nc.dram_tensor("k_row", [B, DLOC], f32, kind="ExternalOutput")
    v_row = nc.dram_tensor("v_row", [B, DLOC], f32, kind="ExternalOutput")

    dbg_outs = {}

    def dbg_tensor(name, shape):
        if dbg:
            dbg_outs[name] = nc.dram_tensor(name, shape, f32, kind="ExternalOutput")

    dbg_tensor("dbg_zT", [P, KC, B])
    dbg_tensor("dbg_qkv", [B, 3 * DLOC])
    dbg_tensor("dbg_qr", [B, DLOC])
    dbg_tensor("dbg_kr", [B, DLOC])
    dbg_tensor("dbg_qTn", [P, 2 * NH])
    dbg_tensor("dbg_exp0", [P, max(SC, 1)])
    dbg_tensor("dbg_rows", [P, 4 * NH])
    dbg_tensor("dbg_sums", [1, 4 * NH])
    dbg_tensor("dbg_ctx", [P, 2 * NH])
    dbg_tensor("dbg_ao", [B, D])
    dbg_tensor("dbg_xmid", [B, D])
    dbg_tensor("dbg_h", [B, D])
    dbg_tensor("dbg_m", [B, FFP])

    with tile.TileContext(nc) as tc:
        with tc.tile_pool(name="wpool", bufs=3) as wpool, \
             tc.tile_pool(name="kvpool", bufs=3) as kvpool, \
             tc.tile_pool(name="bvpool", bufs=2) as bvpool, \
             tc.tile_pool(name="spool", bufs=1) as spool, \
             tc.tile_pool(name="xpool", bufs=2) as xpool, \
             tc.tile_pool(name="cpool", bufs=1) as cpool, \
             tc.tile_pool(name="ps_sc", bufs=2, space="PSUM") as ps_sc, \
             tc.tile_pool(name="ps_ctx", bufs=1, space="PSUM") as ps_ctx, \
             tc.tile_pool(name="ps_vec", bufs=3, space="PSUM") as ps_vec, \
             tc.tile_pool(name="ps_tp", bufs=2, space="PSUM") as ps_tp, \
             tc.tile_pool(name="dram", bufs=1, space="DRAM") as dram:

            # ---- constants ----------------------------------------------
            ident = cpool.tile([P, P], f32, tag="ident")
            make_identity(nc, ident[:])
            ones = cpool.tile([P, 1], f32, tag="ones")
            nc.vector.memset(ones[:], 1.0)
            ones_row = cpool.tile([1, P], f32, tag="ones_row")
            nc.vector.memset(ones_row[:], 1.0)
            eps_c = cpool.tile([P, 1], f32, tag="eps")
            nc.vector.memset(eps_c[:], EPS)
            rq = cpool.tile([B, HD], f32, tag="rq")
            nc.sync.dma_start(rq[:], ropeq.ap())
            rk = cpool.tile([B, HD], f32, tag="rk")
            nc.sync.dma_start(rk[:], ropek.ap())
            x_sb = cpool.tile([B, D], f32, tag="x")
            nc.sync.dma_start(x_sb[:], x_in.ap())
            xT_sb = cpool.tile([P, KC, B], f32, tag="xT")
            nc.sync.dma_start(xT_sb[:], xT_in.ap())
            if SC > 0:
                mask_sb = cpool.tile([P, SC], f32, tag="mask")
                nc.sync.dma_start(mask_sb[:], mask_in.ap())

            def mm_block(psum_ap, lhsT_fn, w_view, k_chunks, n0, n_sz):
                """psum += sum_k lhsT(k).T @ W^T[k-chunk, n0:n0+n_sz] streaming
                weight tiles of up to 8 k-chunks from DRAM."""
                n_tiles = (k_chunks + 7) // 8
                for t in range(n_tiles):
                    nch = min(8, k_chunks - t * 8)
                    wt = wpool.tile([P, 8, 512], f32, tag="w")
                    nc.sync.dma_start(
                        wt[:, :nch, :n_sz],
                        w_view[:, t * 8:t * 8 + nch, n0:n0 + n_sz])
                    for kc in range(nch):
                        kg = t * 8 + kc
                        nc.tensor.matmul(
                            psum_ap, lhsT=lhsT_fn(kg), rhs=wt[:, kc, :n_sz],
                            start=(kg == 0), stop=(kg == k_chunks - 1))

            # ---- pre-norm (rms) in transposed layout ---------------------
            # zT[p, c, b] = xT[p, c, b] * rsqrt(mean_D(x_b^2) + eps)
            xsq = spool.tile([P, KC, B], f32, tag="xsq")
            nc.vector.tensor_tensor(xsq[:], xT_sb[:], xT_sb[:], Alu.mult)
            xsq_red = spool.tile([P, B], f32, tag="xsqred")
            nc.vector.reduce_sum(xsq_red[:], xsq[:].rearrange("p c b -> p b c"),
                                 axis=AX.X)
            ps_msq = ps_tp.tile([1, B], f32, tag="tp")
            nc.tensor.matmul(ps_msq[:], lhsT=ones[:], rhs=xsq_red[:],
                             start=True, stop=True)
            rstd_t = spool.tile([1, B], f32, tag="rstd")
            nc.scalar.activation(rstd_t[:], ps_msq[:], Act.Sqrt,
                                 bias=eps_c[:1], scale=1.0 / D)
            rstd = spool.tile([1, B], f32, tag="rstd2")
            nc.vector.reciprocal(rstd[:], rstd_t[:])
            # replicate rstd across all 128 partitions via ones-row outer product
            ps_rbc = ps_tp.tile([P, B], f32, tag="tp")
            nc.tensor.matmul(ps_rbc[:], lhsT=ones_row[:], rhs=rstd[:],
                             start=True, stop=True)
            zT = spool.tile([P, KC, B], f32, tag="zT")
            nc.vector.tensor_tensor(zT[:], xT_sb[:],
                                    ps_rbc[:, None, :].to_broadcast((P, KC, B)),
                                    Alu.mult)
            if dbg:
                nc.sync.dma_start(dbg_outs["dbg_zT"].ap(), zT[:])

            # ---- QKV: [B, 1536] = z @ WqkvT ------------------------------
            wqkv_view = wqkvT.ap().rearrange("(o p) n -> p o n", p=P)
            qkv_sb = spool.tile([B, 3 * DLOC], f32, tag="qkv")
            for nb in range(3):
                ps_q = ps_vec.tile([B, 512], f32, tag="vec")
                mm_block(ps_q[:], lambda k: zT[:, k, :], wqkv_view, KC, nb * 512, 512)
                nc.vector.tensor_copy(qkv_sb[:, nb * 512:(nb + 1) * 512], ps_q[:])
            if dbg:
                nc.sync.dma_start(dbg_outs["dbg_qkv"].ap(), qkv_sb[:])

            # ---- RoPE on q and k (b-part layout) -------------------------
            def rope(dst, src_ap, rvec):
                # dst[b,h,:64] = s1*cos - s2*sin ; dst[b,h,64:] = s2*cos + s1*sin
                s1 = src_ap[:, :, 0:64]
                s2 = src_ap[:, :, 64:128]
                cosb = rvec[:, None, 0:64].to_broadcast((B, NH, 64))
                sinb = rvec[:, None, 64:128].to_broadcast((B, NH, 64))
                tmp = spool.tile([B, NH, 64], f32, tag="ropetmp")
                d1 = dst[:].rearrange("b (h d) -> b h d", d=HD)[:, :, 0:64]
                d2 = dst[:].rearrange("b (h d) -> b h d", d=HD)[:, :, 64:128]
                nc.vector.tensor_tensor(d1, s1, cosb, Alu.mult)
                nc.vector.tensor_tensor(tmp[:], s2, sinb, Alu.mult)
                nc.vector.tensor_sub(d1, d1, tmp[:])
                nc.vector.tensor_tensor(d2, s2, cosb, Alu.mult)
                nc.vector.tensor_tensor(tmp[:], s1, sinb, Alu.mult)
                nc.vector.tensor_add(d2, d2, tmp[:])

            q_view = qkv_sb[:, 0:DLOC].rearrange("b (h d) -> b h d", d=HD)
            k_view = qkv_sb[:, DLOC:2 * DLOC].rearrange("b (h d) -> b h d", d=HD)
            qr = spool.tile([B, DLOC], f32, tag="qr")     # scaled by 1/sqrt(HD)
            kr = spool.tile([B, DLOC], f32, tag="kr")
            rope(qr, q_view, rq)
            rope(kr, k_view, rk)
            nc.sync.dma_start(k_row.ap(), kr[:])
            nc.sync.dma_start(v_row.ap(), qkv_sb[:, 2 * DLOC:3 * DLOC])
            if dbg:
                nc.sync.dma_start(dbg_outs["dbg_qr"].ap(), qr[:])
                nc.sync.dma_start(dbg_outs["dbg_kr"].ap(), kr[:])

            # ---- per-head transposes: [2,128] -> [128,2] ------------------
            # col layout everywhere below: j = 2*h + b
            qTn = spool.tile([P, 2 * NH], f32, tag="qTn")
            kTn = spool.tile([P, 2 * NH], f32, tag="kTn")
            vTn = spool.tile([P, 2 * NH], f32, tag="vTn")
            for h in range(NH):
                for src, dst in ((qr[:, h * HD:(h + 1) * HD], qTn),
                                 (kr[:, h * HD:(h + 1) * HD], kTn),
                                 (qkv_sb[:, 2 * DLOC + h * HD:2 * DLOC + (h + 1) * HD], vTn)):
                    ps_t = ps_tp.tile([P, B], f32, tag="tp")
                    nc.tensor.transpose(ps_t[:], src, ident[:B, :B])
                    nc.vector.tensor_copy(dst[:, 2 * h:2 * h + 2], ps_t[:])
            if dbg:
                nc.sync.dma_start(dbg_outs["dbg_qTn"].ap(), qTn[:])

            # ---- attention over the cache --------------------------------
            # rows_all cols 0..7: per-(b,h) exp-sums partial (per partition)
            # rows_all cols 8..15: q^T (x) k^T elementwise (new-token score parts)
            rows_all = spool.tile([P, 4 * NH], f32, tag="rows")
            if SC == 0:
                nc.vector.memset(rows_all[:, 0:2 * NH], 0.0)
            nc.vector.tensor_tensor(rows_all[:, 2 * NH:4 * NH], qTn[:], kTn[:],
                                    Alu.mult)

            ctx_ps = ps_ctx.tile([P, 2 * NH], f32, tag="ctx")
            if SC > 0:
                kT_ap = kT_in.ap()
                v_view = v_in.ap().rearrange("b (so p) h d -> b p so h d", p=P)
                NBLK = (SC + SBLK - 1) // SBLK
                for h in range(NH):
                    for b in range(B):
                        j = 2 * h + b
                        sc_ps = ps_sc.tile([P, 64], f32, tag="score")
                        for blk in range(NBLK):
                            nch = min(SBLK, SC - blk * SBLK)
                            kt_t = kvpool.tile([P, SBLK * P], f32, tag="kt")
                            nc.sync.dma_start(
                                kt_t[:, :nch * P],
                                kT_ap[b, h, :, blk * SBLK * P: blk * SBLK * P + nch * P])
                            for f in range(nch):
                                c = blk * SBLK + f
                                nc.tensor.matmul(
                                    sc_ps[:, c:c + 1],
                                    lhsT=kt_t[:, f * P:(f + 1) * P],
                                    rhs=qTn[:, j:j + 1],
                                    start=True, stop=True)
                        exp_sb = xpool.tile([P, 64], f32, tag="exp")
                        nc.scalar.activation(exp_sb[:, :SC], sc_ps[:, :SC], Act.Exp)
                        nc.vector.tensor_tensor(exp_sb[:, :SC], exp_sb[:, :SC],
                                                mask_sb[:], Alu.mult)
                        nc.vector.reduce_sum(rows_all[:, j:j + 1],
                                             exp_sb[:, :SC], axis=AX.X)
                        if dbg and j == 0:
                            nc.sync.dma_start(dbg_outs["dbg_exp0"].ap(),
                                              exp_sb[:, :SC])
                        for blk in range(NBLK):
                            nch = min(SBLK, SC - blk * SBLK)
                            v_t = kvpool.tile([P, SBLK, HD], f32, tag="v")
                            nc.sync.dma_start(
                                v_t[:, :nch, :],
                                v_view[b, :, blk * SBLK: blk * SBLK + nch, h, :])
                            for f in range(nch):
                                c = blk * SBLK + f
                                nc.tensor.matmul(
                                    ctx_ps[:, j:j + 1],
                                    lhsT=v_t[:, f, :],
                                    rhs=exp_sb[:, c:c + 1],
                                    start=(c == 0), stop=(c == SC - 1))
            if dbg:
                nc.sync.dma_start(dbg_outs["dbg_rows"].ap(), rows_all[:])

            # ---- softmax denominators + new-token term -------------------
            ps_sums = ps_tp.tile([1, 4 * NH], f32, tag="tp")
            nc.tensor.matmul(ps_sums[:], lhsT=ones[:], rhs=rows_all[:],
                             start=True, stop=True)
            # ed cols 0..7: e_new = exp(q.k_new); cols 8..15: 1/denominator
            ed = spool.tile([1, 4 * NH], f32, tag="ed")
            nc.scalar.activation(ed[:, 0:2 * NH], ps_sums[:, 2 * NH:4 * NH], Act.Exp)
            denom = spool.tile([1, 2 * NH], f32, tag="denom")
            nc.vector.tensor_add(denom[:], ps_sums[:, 0:2 * NH], ed[:, 0:2 * NH])
            nc.vector.reciprocal(ed[:, 2 * NH:4 * NH], denom[:])
            # replicate (e_new | recip) across partitions
            ps_bc = ps_tp.tile([P, 4 * NH], f32, tag="tp")
            nc.tensor.matmul(ps_bc[:], lhsT=ones_row[:], rhs=ed[:],
                             start=True, stop=True)
            if dbg:
                sums_sb = spool.tile([1, 4 * NH], f32, tag="sums_sb")
                nc.vector.tensor_copy(sums_sb[:], ps_sums[:])
                nc.sync.dma_start(dbg_outs["dbg_sums"].ap(), sums_sb[:])

            # ctx = (ctx_cache + v_new * e_new) / denom     [128, 2*NH]
            ctx_sb = spool.tile([P, 2 * NH], f32, tag="ctx_sb")
            nc.vector.tensor_tensor(ctx_sb[:], vTn[:], ps_bc[:, 0:2 * NH], Alu.mult)
            if SC > 0:
                nc.vector.tensor_add(ctx_sb[:], ctx_sb[:], ctx_ps[:])
            nc.vector.tensor_tensor(ctx_sb[:], ctx_sb[:], ps_bc[:, 2 * NH:4 * NH],
                                    Alu.mult)
            if dbg:
                nc.sync.dma_start(dbg_outs["dbg_ctx"].ap(), ctx_sb[:])

            # ---- out_proj partial + AllReduce ----------------------------
            op_view = opT.ap().rearrange("(o p) n -> p o n", p=P)   # [128,4,4096]
            ao_sb = bvpool.tile([B, D], f32, tag="bv")
            for nb in range(8):
                ps_o = ps_vec.tile([B, 512], f32, tag="vec")
                mm_block(ps_o[:], lambda k: ctx_sb[:, 2 * k:2 * k + 2],
                         op_view, NH, nb * 512, 512)
                nc.vector.tensor_copy(ao_sb[:, nb * 512:(nb + 1) * 512], ps_o[:])
            if dbg:
                nc.sync.dma_start(dbg_outs["dbg_ao"].ap(), ao_sb[:])

            ar1_in = dram.tile([B, D], f32, tag="ar1i")
            ar1_out = dram.tile([B, D], f32, tag="ar1o")
            nc.sync.dma_start(ar1_in[:], ao_sb[:])
            nc.gpsimd.collective_compute(
                "AllReduce", Alu.add,
                replica_groups=[list(range(N_CORES))],
                ins=[ar1_in.opt()], outs=[ar1_out.opt()])
            ar1_sb = bvpool.tile([B, D], f32, tag="bv")
            nc.sync.dma_start(ar1_sb[:], ar1_out[:])

            # ---- residual + post-norm ------------------------------------
            x_mid = bvpool.tile([B, D], f32, tag="bv")
            nc.vector.tensor_add(x_mid[:], x_sb[:], ar1_sb[:])
            if dbg:
                nc.sync.dma_start(dbg_outs["dbg_xmid"].ap(), x_mid[:])
            hsq = bvpool.tile([B, D], f32, tag="bv")
            nc.scalar.activation(hsq[:], x_mid[:], Act.Square)
            hsq_red = spool.tile([B, 1], f32, tag="hsqred")
            nc.vector.reduce_sum(hsq_red[:], hsq[:], axis=AX.X)
            hstd_t = spool.tile([B, 1], f32, tag="hstd")
            nc.scalar.activation(hstd_t[:], hsq_red[:], Act.Sqrt,
                                 bias=eps_c[:B], scale=1.0 / D)
            hstd = spool.tile([B, 1], f32, tag="hstd2")
            nc.vector.reciprocal(hstd[:], hstd_t[:])
            h_sb = bvpool.tile([B, D], f32, tag="bv")
            nc.vector.tensor_tensor(h_sb[:], x_mid[:],
                                    hstd[:].to_broadcast((B, D)), Alu.mult)
            if dbg:
                nc.sync.dma_start(dbg_outs["dbg_h"].ap(), h_sb[:])

            hT = spool.tile([P, KC, B], f32, tag="hT")
            for c in range(KC):
                ps_t = ps_tp.tile([P, B], f32, tag="tp")
                nc.tensor.transpose(ps_t[:], h_sb[:, c * P:(c + 1) * P],
                                    ident[:B, :B])
                nc.vector.tensor_copy(hT[:, c, :], ps_t[:])

            # ---- MLP: m = silu(h@l1T) * (h@l2T) --------------------------
            l1_view = l1T.ap().rearrange("(o p) n -> p o n", p=P)
            l2_view = l2T.ap().rearrange("(o p) n -> p o n", p=P)
            m_sb = spool.tile([B, FFP], f32, tag="ff_m")
            gs_sb = spool.tile([B, FFP], f32, tag="ff_g")
            for nb, (n0, n_sz) in enumerate(((0, 512), (512, 512), (1024, 384))):
                ps_g = ps_vec.tile([B, 512], f32, tag="vec")
                mm_block(ps_g[:, :n_sz], lambda k: hT[:, k, :], l1_view, KC, n0, n_sz)
                ps_g2 = ps_vec.tile([B, 512], f32, tag="vec")
                mm_block(ps_g2[:, :n_sz], lambda k: hT[:, k, :], l2_view, KC, n0, n_sz)
                # silu(g) = g * sigmoid(g)  (Silu isn't in the simulator)
                nc.scalar.activation(gs_sb[:, n0:n0 + n_sz], ps_g[:, :n_sz],
                                     Act.Sigmoid)
                nc.vector.tensor_tensor(gs_sb[:, n0:n0 + n_sz],
                                        gs_sb[:, n0:n0 + n_sz],
                                        ps_g[:, :n_sz], Alu.mult)
                nc.vector.tensor_tensor(m_sb[:, n0:n0 + n_sz],
                                        gs_sb[:, n0:n0 + n_sz],
                                        ps_g2[:, :n_sz], Alu.mult)
            if dbg:
                nc.sync.dma_start(dbg_outs["dbg_m"].ap(), m_sb[:])

            mT = spool.tile([P, FFC, B], f32, tag="mT")
            for c in range(FFC):
                ps_t = ps_tp.tile([P, B], f32, tag="tp")
                nc.tensor.transpose(ps_t[:], m_sb[:, c * P:(c + 1) * P],
                                    ident[:B, :B])
                nc.vector.tensor_copy(mT[:, c, :], ps_t[:])

            # ---- l3 partial + AllReduce + final residual -----------------
            l3_view = l3T.ap().rearrange("(o p) n -> p o n", p=P)   # [128,11,4096]
            pre_ar = bvpool.tile([B, D], f32, tag="bv")
            for nb in range(8):
                ps_y = ps_vec.tile([B, 512], f32, tag="vec")
                mm_block(ps_y[:], lambda k: mT[:, k, :], l3_view, FFC, nb * 512, 512)
                nc.vector.tensor_copy(pre_ar[:, nb * 512:(nb + 1) * 512], ps_y[:])

            ar2_in = dram.tile([B, D], f32, tag="ar2i")
            ar2_out = dram.tile([B, D], f32, tag="ar2o")
            nc.sync.dma_start(ar2_in[:], pre_ar[:])
            nc.gpsimd.collective_compute(
                "AllReduce", Alu.add,
                replica_groups=[list(range(N_CORES))],
                ins=[ar2_in.opt()], outs=[ar2_out.opt()])
            ar2_sb = bvpool.tile([B, D], f32, tag="bv")
            nc.sync.dma_start(ar2_sb[:], ar2_out[:])
            nc.vector.tensor_add(x_mid[:], x_mid[:], ar2_sb[:])
            nc.sync.dma_start(x_out.ap(), x_mid[:])

    nc.compile()
    return nc


def _prep_core_inputs(c, pos, x, k_cache, v_cache, pre_norm_w, post_norm_w,
                      Wqkv_w, out_proj_w, l1_w, l2_w, l3_w, rope_cos, rope_sin):
    SC = (pos + P - 1) // P
    SCP = SC * P
    hs = c * NH
    fs = c * FFL

    cf = lambda a: np.ascontiguousarray(a, dtype=np.float32)

    # Wqkv rows for this core's heads (q, k, v stacked as columns after T),
    # pre-norm weight folded into the contraction dim.
    rows = []
    for g in range(3):
        rows.append(Wqkv_w[g * D + hs * HD: g * D + (hs + NH) * HD, :])
    w_c = np.concatenate(rows, axis=0) * pre_norm_w[None, :]      # [1536, 4096]
    wqkvT = cf(w_c.T)                                             # [4096, 1536]

    opT = cf(out_proj_w[:, hs * HD:(hs + NH) * HD].T)             # [512, 4096]

    l1_c = l1_w[fs:fs + FFL, :] * post_norm_w[None, :]
    l2_c = l2_w[fs:fs + FFL, :] * post_norm_w[None, :]
    l1T = np.zeros((D, FFP), np.float32)
    l2T = np.zeros((D, FFP), np.float32)
    l1T[:, :FFL] = l1_c.T
    l2T[:, :FFL] = l2_c.T
    l3T = np.zeros((FFP, D), np.float32)
    l3T[:FFL, :] = l3_w[:, fs:fs + FFL].T

    scale = 1.0 / math.sqrt(HD)
    cos_h = np.asarray(rope_cos)[pos].astype(np.float32)
    sin_h = np.asarray(rope_sin)[pos].astype(np.float32)
    ropeq = cf(np.tile(np.concatenate([cos_h, sin_h])[None, :] * scale, (B, 1)))
    ropek = cf(np.tile(np.concatenate([cos_h, sin_h])[None, :], (B, 1)))

    xT = cf(x.reshape(B, D).T.reshape(KC, P, B).transpose(1, 0, 2))

    im = dict(
        x=cf(x.reshape(B, D)), xT=xT, wqkvT=wqkvT, opT=cf(opT),
        l1T=l1T, l2T=l2T, l3T=l3T, ropeq=ropeq, ropek=ropek,
    )
    if SC > 0:
        kT = cf(k_cache[:, :SCP, hs:hs + NH, :].transpose(0, 2, 3, 1))
        v = cf(v_cache[:, :SCP, hs:hs + NH, :])
        sidx = (np.arange(SC)[None, :] * P + np.arange(P)[:, None])
        mask = (sidx < pos).astype(np.float32)
        im.update(kT=kT, v=v, mask=cf(mask))
    return im


def kernel(x, position, k_cache, v_cache, pre_norm_w, post_norm_w,
           Wqkv_w, out_proj_w, l1_w, l2_w, l3_w, rope_cos, rope_sin,
           _dbg=False):
    global LAST_RESULT
    _install_ntff_shim()
    from concourse import bass_utils

    x = np.asarray(x)
    k_cache = np.asarray(k_cache)
    v_cache = np.asarray(v_cache)
    args = [np.asarray(a) for a in (pre_norm_w, post_norm_w, Wqkv_w,
                                    out_proj_w, l1_w, l2_w, l3_w,
                                    rope_cos, rope_sin)]
    pos = int(np.asarray(position).reshape(-1)[0])

    key = (pos, _dbg)
    if key not in _NC_CACHE:
        _NC_CACHE[key] = _build(pos, dbg=_dbg)
    nc = _NC_CACHE[key]

    in_maps = [_prep_core_inputs(c, pos, x, k_cache, v_cache, *args)
               for c in range(N_CORES)]
    res = bass_utils.run_bass_kernel_spmd(nc, in_maps,
                                          core_ids=list(range(N_CORES)))
    LAST_RESULT = res

    x_new = res.results[0]["x_out"].reshape(B, 1, D).astype(np.float32)
    k_out = np.array(k_cache, dtype=np.float32, copy=True)
    v_out = np.array(v_cache, dtype=np.float32, copy=True)
    for c in range(N_CORES):
        hs = c * NH
        k_out[:, pos, hs:hs + NH, :] = res.results[c]["k_row"].reshape(B, NH, HD)
        v_out[:, pos, hs:hs + NH, :] = res.results[c]["v_row"].reshape(B, NH, HD)
    return (x_new, k_out, v_out)


# revision 30
# speedup vs baseline: 1.1903x; 1.1903x over previous
"""Trainium2 Bass kernel for a single-token transformer decode block.

Model: B=2, S=8192, H=32, HD=128, D=4096, FF=11008 (gated SiLU MLP), RoPE,
RMSNorm, KV-cache decode at runtime position `pos`.

Sharding (tensor-parallel over heads, 8 cores):
  - core c owns heads 4c..4c+3: Wqkv rows, out_proj columns, k/v cache head
    slices, and an FF shard (1376 padded to 1408) of l1/l2/l3.
  - AllReduce after out_proj and after l3.
  - norms are folded into the weight matrices on the host; RoPE rows at `pos`
    are computed on the host (position is known when kernel() is called, so
    the device program is specialized on it: only ceil(pos/128) KV chunks are
    ever read or attended).

Layout strategy on device (everything fp32):
  - activations live as [B=2, features] on partitions 0..1 ("b-part") except
    where a matmul needs features-on-partitions; those are produced either by
    host-side transposes (weights, xT) or cheap PE transposes (h, m, q/k/v).
  - scores for one (b,h) live as [128 (s within chunk), n_chunks] in PSUM,
    computed by per-chunk matmuls lhsT=K^T[128hd,128s], rhs=q^T[128,1].
  - softmax skips the max-subtraction (scores are O(5) here; exp is safe in
    fp32 and matches the reference's -10000 masking, which underflows to 0).
  - the s==pos entry (new k/v) is handled as a separate rank-1 term.
"""

import math
import os
import sys
import types

import numpy as np

B = 2
S = 8192
H = 32
HD = 128
D = 4096
FF = 11008
P = 128
N_CORES = 8
NH = H // N_CORES            # 4 heads per core
DLOC = NH * HD               # 512
FFL = FF // N_CORES          # 1376
FFP = 1408                   # FFL padded to a multiple of 128
KC = D // P                  # 32 k-chunks of the model dim
FFC = FFP // P               # 11
EPS = 1e-6
SBLK = 16                    # s-chunks (of 128) per KV DMA block = 2048 positions

LAST_RESULT = None           # BassKernelResults of the most recent run
_NC_CACHE = {}


def _install_ntff_shim():
    """Make trace=True work under axon when antenv.axon_hooks is missing."""
    try:
        import antenv.axon_hooks  # noqa: F401
        return
    except ImportError:
        pass
    try:
        from trn_agent_boot.trn_boot import _ntff_profile_via_ctypes
        mod = types.ModuleType("antenv.axon_hooks")
        _hook = [None]
        mod.set_axon_ntff_profile_hook = lambda h: _hook.__setitem__(0, h)
        mod.get_axon_ntff_profile_hook = lambda: _hook[0]
        sys.modules["antenv.axon_hooks"] = mod
        mod.set_axon_ntff_profile_hook(
            _ntff_profile_via_ctypes("/opt/axon/libaxon_pjrt.so"))
    except Exception:
        pass


def _build(pos: int, dbg: bool = False):
    import concourse.mybir as mybir
    import concourse.tile as tile
    from concourse import bacc
    from concourse.masks import make_identity

    f32 = mybir.dt.float32
    f32r = mybir.dt.float32r     # 4-byte fp32, single-pass PE mode (~1.6e-4)
    Alu = mybir.AluOpType
    Act = mybir.ActivationFunctionType
    AX = mybir.AxisListType

    SC = (pos + P - 1) // P          # 128-chunks covering cache positions [0, pos)
    SCP = SC * P

    nc = bacc.Bacc("TRN2", target_bir_lowering=False, debug=False,
                   enable_asserts=False, num_devices=N_CORES)

    # ---- I/O -------------------------------------------------------------
    x_in = nc.dram_tensor("x", [B, D], f32, kind="ExternalInput")
    xT_in = nc.dram_tensor("xT", [P, KC, B], f32, kind="ExternalInput")
    wqkvT = nc.dram_tensor("wqkvT", [D, 3 * DLOC], f32, kind="ExternalInput")
    opT = nc.dram_tensor("opT", [DLOC, D], f32r, kind="ExternalInput")
    l1T = nc.dram_tensor("l1T", [D, FFP], f32r, kind="ExternalInput")
    l2T = nc.dram_tensor("l2T", [D, FFP], f32r, kind="ExternalInput")
    l3T = nc.dram_tensor("l3T", [FFP, D], f32r, kind="ExternalInput")
    ropeq = nc.dram_tensor("ropeq", [B, HD], f32, kind="ExternalInput")
    ropek = nc.dram_tensor("ropek", [B, HD], f32, kind="ExternalInput")
    if SC > 0:
        kT_in = nc.dram_tensor("kT", [B, NH, HD, SCP], f32r, kind="ExternalInput")
        v_in = nc.dram_tensor("v", [B, SCP, NH, HD], f32r, kind="ExternalInput")
        mask_in = nc.dram_tensor("mask", [P, SC], f32, kind="ExternalInput")

    x_out = nc.dram_tensor("x_out", [B, D], f32, kind="ExternalOutput")
    k_row = nc.dram_tensor("k_row", [B, DLOC], f32, kind="ExternalOutput")
    v_row = nc.dram_tensor("v_row", [B, DLOC], f32, kind="ExternalOutput")

    dbg_outs = {}

    def dbg_tensor(name, shape):
        if dbg:
            dbg_outs[name] = nc.dram_tensor(name, shape, f32, kind="ExternalOutput")

    dbg_tensor("dbg_zT", [P, KC, B])
    dbg_tensor("dbg_qkv", [B, 3 * DLOC])
    dbg_tensor("dbg_qr", [B, DLOC])
    dbg_tensor("dbg_kr", [B, DLOC])
    dbg_tensor("dbg_qTn", [P, 2 * NH])
    dbg_tensor("dbg_exp0", [P, max(SC, 1)])
    dbg_tensor("dbg_rows", [P, 4 * NH])
    dbg_tensor("dbg_sums", [1, 4 * NH])
    dbg_tensor("dbg_ctx", [P, 2 * NH])
    dbg_tensor("dbg_ao", [B, D])
    dbg_tensor("dbg_xmid", [B, D])
    dbg_tensor("dbg_h", [P, KC, B])
    dbg_tensor("dbg_m", [B, FFP])

    with tile.TileContext(nc) as tc:
        with tc.tile_pool(name="wpool", bufs=3) as wpool, \
             tc.tile_pool(name="kvpool", bufs=3) as kvpool, \
             tc.tile_pool(name="bvpool", bufs=2) as bvpool, \
             tc.tile_pool(name="spool", bufs=1) as spool, \
             tc.tile_pool(name="xpool", bufs=2) as xpool, \
             tc.tile_pool(name="cpool", bufs=1) as cpool, \
             tc.tile_pool(name="ps_sc", bufs=2, space="PSUM") as ps_sc, \
             tc.tile_pool(name="ps_ctx", bufs=1, space="PSUM") as ps_ctx, \
             tc.tile_pool(name="ps_vec", bufs=3, space="PSUM") as ps_vec, \
             tc.tile_pool(name="ps_tp", bufs=2, space="PSUM") as ps_tp, \
             tc.tile_pool(name="dram", bufs=1, space="DRAM") as dram:

            # ---- constants ----------------------------------------------
            ident = cpool.tile([P, P], f32, tag="ident")
            make_identity(nc, ident[:])
            ones = cpool.tile([P, 1], f32, tag="ones")
            nc.vector.memset(ones[:], 1.0)
            ones_row = cpool.tile([1, P], f32, tag="ones_row")
            nc.vector.memset(ones_row[:], 1.0)
            eps_c = cpool.tile([P, 1], f32, tag="eps")
            nc.vector.memset(eps_c[:], EPS)
            rq = cpool.tile([B, HD], f32, tag="rq")
            nc.sync.dma_start(rq[:], ropeq.ap())
            rk = cpool.tile([B, HD], f32, tag="rk")
            nc.sync.dma_start(rk[:], ropek.ap())
            x_sb = cpool.tile([B, D], f32, tag="x")
            nc.sync.dma_start(x_sb[:], x_in.ap())
            xT_sb = cpool.tile([P, KC, B], f32, tag="xT")
            nc.sync.dma_start(xT_sb[:], xT_in.ap())
            if SC > 0:
                mask_sb = cpool.tile([P, SC], f32, tag="mask")
                nc.sync.dma_start(mask_sb[:], mask_in.ap())

            def mm_block(psum_ap, lhsT_fn, w_view, k_chunks, n0, n_sz, wdt=f32r):
                """psum += sum_k lhsT(k).T @ W^T[k-chunk, n0:n0+n_sz] streaming
                weight tiles of up to 8 k-chunks from DRAM."""
                n_tiles = (k_chunks + 7) // 8
                for t in range(n_tiles):
                    nch = min(8, k_chunks - t * 8)
                    wt = wpool.tile([P, 8, 512], wdt, tag="w")
                    nc.sync.dma_start(
                        wt[:, :nch, :n_sz],
                        w_view[:, t * 8:t * 8 + nch, n0:n0 + n_sz])
                    for kc in range(nch):
                        kg = t * 8 + kc
                        nc.tensor.matmul(
                            psum_ap, lhsT=lhsT_fn(kg), rhs=wt[:, kc, :n_sz],
                            start=(kg == 0), stop=(kg == k_chunks - 1))

            # ---- pre-norm (rms) in transposed layout ---------------------
            # zT[p, c, b] = xT[p, c, b] * rsqrt(mean_D(x_b^2) + eps)
            xsq = spool.tile([P, KC, B], f32, tag="xsq")
            nc.vector.tensor_tensor(xsq[:], xT_sb[:], xT_sb[:], Alu.mult)
            xsq_red = spool.tile([P, B], f32, tag="xsqred")
            nc.vector.reduce_sum(xsq_red[:], xsq[:].rearrange("p c b -> p b c"),
                                 axis=AX.X)
            ps_msq = ps_tp.tile([1, B], f32, tag="tp")
            nc.tensor.matmul(ps_msq[:], lhsT=ones[:], rhs=xsq_red[:],
                             start=True, stop=True)
            rstd_t = spool.tile([1, B], f32, tag="rstd")
            nc.scalar.activation(rstd_t[:], ps_msq[:], Act.Sqrt,
                                 bias=eps_c[:1], scale=1.0 / D)
            rstd = spool.tile([1, B], f32, tag="rstd2")
            nc.vector.reciprocal(rstd[:], rstd_t[:])
            # replicate rstd across all 128 partitions via ones-row outer product
            ps_rbc = ps_tp.tile([P, B], f32, tag="tp")
            nc.tensor.matmul(ps_rbc[:], lhsT=ones_row[:], rhs=rstd[:],
                             start=True, stop=True)
            zT = spool.tile([P, KC, B], f32, tag="zT")
            nc.vector.tensor_tensor(zT[:], xT_sb[:],
                                    ps_rbc[:, None, :].to_broadcast((P, KC, B)),
                                    Alu.mult)
            if dbg:
                nc.sync.dma_start(dbg_outs["dbg_zT"].ap(), zT[:])

            # ---- QKV: [B, 1536] = z @ WqkvT ------------------------------
            wqkv_view = wqkvT.ap().rearrange("(o p) n -> p o n", p=P)
            qkv_sb = spool.tile([B, 3 * DLOC], f32, tag="qkv")
            for nb in range(3):
                ps_q = ps_vec.tile([B, 512], f32, tag="vec")
                mm_block(ps_q[:], lambda k: zT[:, k, :], wqkv_view, KC,
                         nb * 512, 512, wdt=f32)
                nc.vector.tensor_copy(qkv_sb[:, nb * 512:(nb + 1) * 512], ps_q[:])
            if dbg:
                nc.sync.dma_start(dbg_outs["dbg_qkv"].ap(), qkv_sb[:])

            # ---- RoPE on q and k (b-part layout) -------------------------
            def rope(dst, src_ap, rvec):
                # dst[b,h,:64] = s1*cos - s2*sin ; dst[b,h,64:] = s2*cos + s1*sin
                s1 = src_ap[:, :, 0:64]
                s2 = src_ap[:, :, 64:128]
                cosb = rvec[:, None, 0:64].to_broadcast((B, NH, 64))
                sinb = rvec[:, None, 64:128].to_broadcast((B, NH, 64))
                tmp = spool.tile([B, NH, 64], f32, tag="ropetmp")
                d1 = dst[:].rearrange("b (h d) -> b h d", d=HD)[:, :, 0:64]
                d2 = dst[:].rearrange("b (h d) -> b h d", d=HD)[:, :, 64:128]
                nc.vector.tensor_tensor(d1, s1, cosb, Alu.mult)
                nc.vector.tensor_tensor(tmp[:], s2, sinb, Alu.mult)
                nc.vector.tensor_sub(d1, d1, tmp[:])
                nc.vector.tensor_tensor(d2, s2, cosb, Alu.mult)
                nc.vector.tensor_tensor(tmp[:], s1, sinb, Alu.mult)
                nc.vector.tensor_add(d2, d2, tmp[:])

            q_view = qkv_sb[:, 0:DLOC].rearrange("b (h d) -> b h d", d=HD)
            k_view = qkv_sb[:, DLOC:2 * DLOC].rearrange("b (h d) -> b h d", d=HD)
            qr = spool.tile([B, DLOC], f32, tag="qr")     # scaled by 1/sqrt(HD)
            kr = spool.tile([B, DLOC], f32, tag="kr")
            rope(qr, q_view, rq)
            rope(kr, k_view, rk)
            nc.sync.dma_start(k_row.ap(), kr[:])
            nc.sync.dma_start(v_row.ap(), qkv_sb[:, 2 * DLOC:3 * DLOC])
            if dbg:
                nc.sync.dma_start(dbg_outs["dbg_qr"].ap(), qr[:])
                nc.sync.dma_start(dbg_outs["dbg_kr"].ap(), kr[:])

            # ---- per-head transposes: [2,128] -> [128,2] ------------------
            # col layout everywhere below: j = 2*h + b
            qTn = spool.tile([P, 2 * NH], f32r, tag="qTn")
            kTn = spool.tile([P, 2 * NH], f32, tag="kTn")
            vTn = spool.tile([P, 2 * NH], f32, tag="vTn")
            for h in range(NH):
                for src, dst in ((qr[:, h * HD:(h + 1) * HD], qTn),
                                 (kr[:, h * HD:(h + 1) * HD], kTn),
                                 (qkv_sb[:, 2 * DLOC + h * HD:2 * DLOC + (h + 1) * HD], vTn)):
                    ps_t = ps_tp.tile([P, B], f32, tag="tp")
                    nc.tensor.transpose(ps_t[:], src, ident[:B, :B])
                    nc.vector.tensor_copy(dst[:, 2 * h:2 * h + 2], ps_t[:])
            if dbg:
                nc.sync.dma_start(dbg_outs["dbg_qTn"].ap(), qTn[:].bitcast(f32))

            # ---- attention over the cache --------------------------------
            # rows_all cols 0..7: per-(b,h) exp-sums partial (per partition)
            # rows_all cols 8..15: q^T (x) k^T elementwise (new-token score parts)
            rows_all = spool.tile([P, 4 * NH], f32, tag="rows")
            if SC == 0:
                nc.vector.memset(rows_all[:, 0:2 * NH], 0.0)
            nc.vector.tensor_tensor(rows_all[:, 2 * NH:4 * NH], qTn[:], kTn[:],
                                    Alu.mult)

            # ctxT_sb collects the cache-attention context, transposed to
            # [hd-on-partitions, col j]
            ctxT_sb = spool.tile([P, 2 * NH], f32, tag="ctxT")
            if SC > 0:
                kT_ap = kT_in.ap()
                v_view = v_in.ap().rearrange("b (so p) h d -> b p so h d", p=P)
                NBLK = (SC + SBLK - 1) // SBLK
                for h in range(NH):
                    for b in range(B):
                        j = 2 * h + b
                        # scores: out[:, c, :] = K_chunk^T q-pair; real col = b
                        # (f32r needs a moving free dim >= 2)
                        sc_ps = ps_sc.tile([P, 64, 2], f32, tag="score")
                        for blk in range(NBLK):
                            nch = min(SBLK, SC - blk * SBLK)
                            kt_t = kvpool.tile([P, SBLK * P], f32r, tag="kt")
                            nc.sync.dma_start(
                                kt_t[:, :nch * P],
                                kT_ap[b, h, :, blk * SBLK * P: blk * SBLK * P + nch * P])
                            for f in range(nch):
                                c = blk * SBLK + f
                                nc.tensor.matmul(
                                    sc_ps[:, c, :],
                                    lhsT=kt_t[:, f * P:(f + 1) * P],
                                    rhs=qTn[:, 2 * h:2 * h + 2],
                                    start=True, stop=True)
                        exp_sb = xpool.tile([P, 64], f32r, tag="exp")
                        nc.scalar.activation(exp_sb[:, :SC], sc_ps[:, :SC, b],
                                             Act.Exp)
                        nc.vector.tensor_tensor(exp_sb[:, :SC], exp_sb[:, :SC],
                                                mask_sb[:], Alu.mult)
                        nc.vector.reduce_sum(rows_all[:, j:j + 1],
                                             exp_sb[:, :SC], axis=AX.X)
                        if dbg and j == 0:
                            nc.sync.dma_start(dbg_outs["dbg_exp0"].ap(),
                                              exp_sb[:, :SC].bitcast(f32))
                        # ctx row: exp is the 1-col stationary, V chunks stream
                        ctx_ps = ps_ctx.tile([1, HD], f32, tag="ctx")
                        for blk in range(NBLK):
                            nch = min(SBLK, SC - blk * SBLK)
                            v_t = kvpool.tile([P, SBLK, HD], f32r, tag="v")
                            nc.sync.dma_start(
                                v_t[:, :nch, :],
                                v_view[b, :, blk * SBLK: blk * SBLK + nch, h, :])
                            for f in range(nch):
                                c = blk * SBLK + f
                                nc.tensor.matmul(
                                    ctx_ps[:],
                                    lhsT=exp_sb[:, c:c + 1],
                                    rhs=v_t[:, f, :],
                                    start=(c == 0), stop=(c == SC - 1))
                        crow = xpool.tile([1, HD], f32, tag="crow")
                        nc.vector.tensor_copy(crow[:], ctx_ps[:])
                        ps_ct = ps_tp.tile([P, B], f32, tag="tp")
                        nc.tensor.transpose(ps_ct[:, :1], crow[:], ident[:1, :1])
                        nc.vector.tensor_copy(ctxT_sb[:, j:j + 1], ps_ct[:, :1])
            if dbg:
                nc.sync.dma_start(dbg_outs["dbg_rows"].ap(), rows_all[:])

            # ---- softmax denominators + new-token term -------------------
            ps_sums = ps_tp.tile([1, 4 * NH], f32, tag="tp")
            nc.tensor.matmul(ps_sums[:], lhsT=ones[:], rhs=rows_all[:],
                             start=True, stop=True)
            # ed cols 0..7: e_new = exp(q.k_new); cols 8..15: 1/denominator
            ed = spool.tile([1, 4 * NH], f32, tag="ed")
            nc.scalar.activation(ed[:, 0:2 * NH], ps_sums[:, 2 * NH:4 * NH], Act.Exp)
            denom = spool.tile([1, 2 * NH], f32, tag="denom")
            nc.vector.tensor_add(denom[:], ps_sums[:, 0:2 * NH], ed[:, 0:2 * NH])
            nc.vector.reciprocal(ed[:, 2 * NH:4 * NH], denom[:])
            # replicate (e_new | recip) across partitions
            ps_bc = ps_tp.tile([P, 4 * NH], f32, tag="tp")
            nc.tensor.matmul(ps_bc[:], lhsT=ones_row[:], rhs=ed[:],
                             start=True, stop=True)
            if dbg:
                sums_sb = spool.tile([1, 4 * NH], f32, tag="sums_sb")
                nc.vector.tensor_copy(sums_sb[:], ps_sums[:])
                nc.sync.dma_start(dbg_outs["dbg_sums"].ap(), sums_sb[:])

            # ctx = (ctx_cache + v_new * e_new) / denom     [128, 2*NH]
            ctx_sb = spool.tile([P, 2 * NH], f32r, tag="ctx_sb")
            nc.vector.tensor_tensor(ctx_sb[:], vTn[:], ps_bc[:, 0:2 * NH], Alu.mult)
            if SC > 0:
                nc.vector.tensor_add(ctx_sb[:], ctx_sb[:], ctxT_sb[:])
            nc.vector.tensor_tensor(ctx_sb[:], ctx_sb[:], ps_bc[:, 2 * NH:4 * NH],
                                    Alu.mult)
            if dbg:
                nc.sync.dma_start(dbg_outs["dbg_ctx"].ap(), ctx_sb[:].bitcast(f32))

            # ---- out_proj partial + AllReduce ----------------------------
            op_view = opT.ap().rearrange("(o p) n -> p o n", p=P)   # [128,4,4096]
            ao_sb = bvpool.tile([B, D], f32, tag="bv")
            for nb in range(8):
                ps_o = ps_vec.tile([B, 512], f32, tag="vec")
                mm_block(ps_o[:], lambda k: ctx_sb[:, 2 * k:2 * k + 2],
                         op_view, NH, nb * 512, 512)
                nc.vector.tensor_copy(ao_sb[:, nb * 512:(nb + 1) * 512], ps_o[:])
            if dbg:
                nc.sync.dma_start(dbg_outs["dbg_ao"].ap(), ao_sb[:])

            ar1_in = dram.tile([B, D], f32, tag="ar1i")
            ar1_out = dram.tile([B, D], f32, tag="ar1o")
            nc.sync.dma_start(ar1_in[:], ao_sb[:])
            nc.gpsimd.collective_compute(
                "AllReduce", Alu.add,
                replica_groups=[list(range(N_CORES))],
                ins=[ar1_in.opt()], outs=[ar1_out.opt()])
            ar1_sb = bvpool.tile([B, D], f32, tag="bv")
            nc.sync.dma_start(ar1_sb[:], ar1_out[:])

            # ---- residual + post-norm (norm done in transposed layout) ---
            x_mid = bvpool.tile([B, D], f32, tag="bv")
            nc.vector.tensor_add(x_mid[:], x_sb[:], ar1_sb[:])
            if dbg:
                nc.sync.dma_start(dbg_outs["dbg_xmid"].ap(), x_mid[:])
            xmT = spool.tile([P, KC, B], f32, tag="xmT")
            for c in range(KC):
                ps_t = ps_tp.tile([P, B], f32, tag="tp")
                nc.tensor.transpose(ps_t[:], x_mid[:, c * P:(c + 1) * P],
                                    ident[:B, :B])
                nc.vector.tensor_copy(xmT[:, c, :], ps_t[:])
            xmsq = spool.tile([P, KC, B], f32, tag="xmsq")
            nc.vector.tensor_tensor(xmsq[:], xmT[:], xmT[:], Alu.mult)
            xmsq_red = spool.tile([P, B], f32, tag="xmsqred")
            nc.vector.reduce_sum(xmsq_red[:],
                                 xmsq[:].rearrange("p c b -> p b c"), axis=AX.X)
            ps_msq2 = ps_tp.tile([1, B], f32, tag="tp")
            nc.tensor.matmul(ps_msq2[:], lhsT=ones[:], rhs=xmsq_red[:],
                             start=True, stop=True)
            hstd_t = spool.tile([1, B], f32, tag="hstd")
            nc.scalar.activation(hstd_t[:], ps_msq2[:], Act.Sqrt,
                                 bias=eps_c[:1], scale=1.0 / D)
            hstd = spool.tile([1, B], f32, tag="hstd2")
            nc.vector.reciprocal(hstd[:], hstd_t[:])
            ps_hbc = ps_tp.tile([P, B], f32, tag="tp")
            nc.tensor.matmul(ps_hbc[:], lhsT=ones_row[:], rhs=hstd[:],
                             start=True, stop=True)
            hT = spool.tile([P, KC, B], f32r, tag="hT")
            nc.vector.tensor_tensor(hT[:], xmT[:],
                                    ps_hbc[:, None, :].to_broadcast((P, KC, B)),
                                    Alu.mult)
            if dbg:
                # hT is f32r; round-trip through fp32 for the debug dump
                hT32 = spool.tile([P, KC, B], f32, tag="hT32")
                nc.vector.tensor_copy(hT32[:], hT[:])
                nc.sync.dma_start(dbg_outs["dbg_h"].ap(), hT32[:])

            # ---- MLP: m = silu(h@l1T) * (h@l2T) --------------------------
            l1_view = l1T.ap().rearrange("(o p) n -> p o n", p=P)
            l2_view = l2T.ap().rearrange("(o p) n -> p o n", p=P)
            m_sb = spool.tile([B, FFP], f32, tag="ff_m")
            gs_sb = spool.tile([B, FFP], f32, tag="ff_g")
            for nb, (n0, n_sz) in enumerate(((0, 512), (512, 512), (1024, 384))):
                ps_g = ps_vec.tile([B, 512], f32, tag="vec")
                mm_block(ps_g[:, :n_sz], lambda k: hT[:, k, :], l1_view, KC, n0, n_sz)
                ps_g2 = ps_vec.tile([B, 512], f32, tag="vec")
                mm_block(ps_g2[:, :n_sz], lambda k: hT[:, k, :], l2_view, KC, n0, n_sz)
                # silu(g) = g * sigmoid(g)  (Silu isn't in the simulator)
                nc.scalar.activation(gs_sb[:, n0:n0 + n_sz], ps_g[:, :n_sz],
                                     Act.Sigmoid)
                nc.vector.tensor_tensor(gs_sb[:, n0:n0 + n_sz],
                                        gs_sb[:, n0:n0 + n_sz],
                                        ps_g[:, :n_sz], Alu.mult)
                nc.vector.tensor_tensor(m_sb[:, n0:n0 + n_sz],
                                        gs_sb[:, n0:n0 + n_sz],
                                        ps_g2[:, :n_sz], Alu.mult)
            if dbg:
                nc.sync.dma_start(dbg_outs["dbg_m"].ap(), m_sb[:])

            mT = spool.tile([P, FFC, B], f32r, tag="mT")
            for c in range(FFC):
                ps_t = ps_tp.tile([P, B], f32, tag="tp")
                nc.tensor.transpose(ps_t[:], m_sb[:, c * P:(c + 1) * P],
                                    ident[:B, :B])
                nc.vector.tensor_copy(mT[:, c, :], ps_t[:])

            # ---- l3 partial + AllReduce + final residual -----------------
            l3_view = l3T.ap().rearrange("(o p) n -> p o n", p=P)   # [128,11,4096]
            pre_ar = bvpool.tile([B, D], f32, tag="bv")
            for nb in range(8):
                ps_y = ps_vec.tile([B, 512], f32, tag="vec")
                mm_block(ps_y[:], lambda k: mT[:, k, :], l3_view, FFC, nb * 512, 512)
                nc.vector.tensor_copy(pre_ar[:, nb * 512:(nb + 1) * 512], ps_y[:])

            ar2_in = dram.tile([B, D], f32, tag="ar2i")
            ar2_out = dram.tile([B, D], f32, tag="ar2o")
            nc.sync.dma_start(ar2_in[:], pre_ar[:])
            nc.gpsimd.collective_compute(
                "AllReduce", Alu.add,
                replica_groups=[list(range(N_CORES))],
                ins=[ar2_in.opt()], outs=[ar2_out.opt()])
            ar2_sb = bvpool.tile([B, D], f32, tag="bv")
            nc.sync.dma_start(ar2_sb[:], ar2_out[:])
            nc.vector.tensor_add(x_mid[:], x_mid[:], ar2_sb[:])
            nc.sync.dma_start(x_out.ap(), x_mid[:])

    nc.compile()
    return nc


def _prep_core_inputs(c, pos, x, k_cache, v_cache, pre_norm_w, post_norm_w,
                      Wqkv_w, out_proj_w, l1_w, l2_w, l3_w, rope_cos, rope_sin):
    SC = (pos + P - 1) // P
    SCP = SC * P
    hs = c * NH
    fs = c * FFL

    cf = lambda a: np.ascontiguousarray(a, dtype=np.float32)

    # Wqkv rows for this core's heads (q, k, v stacked as columns after T),
    # pre-norm weight folded into the contraction dim.
    rows = []
    for g in range(3):
        rows.append(Wqkv_w[g * D + hs * HD: g * D + (hs + NH) * HD, :])
    w_c = np.concatenate(rows, axis=0) * pre_norm_w[None, :]      # [1536, 4096]
    wqkvT = cf(w_c.T)                                             # [4096, 1536]

    opT = cf(out_proj_w[:, hs * HD:(hs + NH) * HD].T)             # [512, 4096]

    l1_c = l1_w[fs:fs + FFL, :] * post_norm_w[None, :]
    l2_c = l2_w[fs:fs + FFL, :] * post_norm_w[None, :]
    l1T = np.zeros((D, FFP), np.float32)
    l2T = np.zeros((D, FFP), np.float32)
    l1T[:, :FFL] = l1_c.T
    l2T[:, :FFL] = l2_c.T
    l3T = np.zeros((FFP, D), np.float32)
    l3T[:FFL, :] = l3_w[:, fs:fs + FFL].T

    scale = 1.0 / math.sqrt(HD)
    cos_h = np.asarray(rope_cos)[pos].astype(np.float32)
    sin_h = np.asarray(rope_sin)[pos].astype(np.float32)
    ropeq = cf(np.tile(np.concatenate([cos_h, sin_h])[None, :] * scale, (B, 1)))
    ropek = cf(np.tile(np.concatenate([cos_h, sin_h])[None, :], (B, 1)))

    xT = cf(x.reshape(B, D).T.reshape(KC, P, B).transpose(1, 0, 2))

    im = dict(
        x=cf(x.reshape(B, D)), xT=xT, wqkvT=wqkvT, opT=cf(opT),
        l1T=l1T, l2T=l2T, l3T=l3T, ropeq=ropeq, ropek=ropek,
    )
    if SC > 0:
        kT = cf(k_cache[:, :SCP, hs:hs + NH, :].transpose(0, 2, 3, 1))
        v = cf(v_cache[:, :SCP, hs:hs + NH, :])
        sidx = (np.arange(SC)[None, :] * P + np.arange(P)[:, None])
        mask = (sidx < pos).astype(np.float32)
        im.update(kT=kT, v=v, mask=cf(mask))
    return im


def kernel(x, position, k_cache, v_cache, pre_norm_w, post_norm_w,
           Wqkv_w, out_proj_w, l1_w, l2_w, l3_w, rope_cos, rope_sin,
           _dbg=False):
    global LAST_RESULT
    _install_ntff_shim()
    from concourse import bass_utils

    x = np.asarray(x)
    k_cache = np.asarray(k_cache)
    v_cache = np.asarray(v_cache)
    args = [np.asarray(a) for a in (pre_norm_w, post_norm_w, Wqkv_w,
                                    out_proj_w, l1_w, l2_w, l3_w,
                                    rope_cos, rope_sin)]
    pos = int(np.asarray(position).reshape(-1)[0])

    key = (pos, _dbg)
    if key not in _NC_CACHE:
        _NC_CACHE[key] = _build(pos, dbg=_dbg)
    nc = _NC_CACHE[key]

    in_maps = [_prep_core_inputs(c, pos, x, k_cache, v_cache, *args)
               for c in range(N_CORES)]
    res = bass_utils.run_bass_kernel_spmd(nc, in_maps,
                                          core_ids=list(range(N_CORES)))
    LAST_RESULT = res

    x_new = res.results[0]["x_out"].reshape(B, 1, D).astype(np.float32)
    k_out = np.array(k_cache, dtype=np.float32, copy=True)
    v_out = np.array(v_cache, dtype=np.float32, copy=True)
    for c in range(N_CORES):
        hs = c * NH
        k_out[:, pos, hs:hs + NH, :] = res.results[c]["k_row"].reshape(B, NH, HD)
        v_out[:, pos, hs:hs + NH, :] = res.results[c]["v_row"].reshape(B, NH, HD)
    return (x_new, k_out, v_out)
